# revision 11
# baseline (speedup 1.0000x reference)
"""Trainium2 Bass kernel for Conv2d(128->256, 3x3, stride 1, pad 1) on
x(32,128,56,56) fp32, data-parallel over batch across 8 NeuronCores.

1-D Winograd F(2,3) along H. For each pair of output rows (tile-row t),
4 transform points are built from padded input rows d0..d3 = xp[2t..2t+3]:
  v0 = d0-d2, v1 = d1+d2, v2 = d2-d1, v3 = d1-d3     (bf16, on DVE/GpSimd)
Per point p and horizontal tap kw, a K=128 matmul accumulates over kw:
  M_p[co, t, w] = sum_kw U_p[kw][ci,co]^T v_p[ci, t, w+kw]
with host-transformed weights (g = weight[:, :, kh, kw] over kh):
  U0 = g0, U1 = (g0+g1+g2)/2, U2 = (g0-g1+g2)/2, U3 = g2
Inverse transform (engines share the drain):
  out[2t]   = M0+M1+M2+bias   out[2t+1] = M1-M2-M3+bias
This needs 12 matmul-groups of 392 free-dim per (img, cob, 7 tile-rows)
= 150,528 PE rows/core vs 225,792 for direct conv (1.5x fewer).
Matmuls in bf16 (1 cyc/row), fp32 PSUM accumulate; output stored bf16
and upcast on host (absmax rel err ~4e-3, limit 2e-2).
"""
import numpy as np

N_CORES = 8
N_PER_CORE = 4          # 32 images / 8 cores
C_IN, C_OUT, K = 128, 256, 3
H = W = 56
HP = WP = 58            # padded
TR = H // 2             # 28 Winograd tile-rows (2 output rows each)
TG = 7                  # tile-rows per chunk
N_CHUNK = TR // TG      # 4 chunks per (img, cob)
NF = TG * W             # 392 matmul free dim / PSUM cols
VW = TR * WP            # 1624 cols per transform point

_compiled = {}


def _patch_ldw_opt():
    # walrus ships with --enable-ldw-opt=false hardcoded; enabling the
    # weight-load optimization is verified correct for this kernel and
    # slightly faster. Idempotent process-level patch.
    import concourse.bass_utils as bu

    if getattr(bu.run_command, "_ldw_patched", False):
        return
    orig = bu.run_command

    def patched(argv, **kw):
        argv = ["--enable-ldw-opt=true" if a == "--enable-ldw-opt=false" else a
                for a in argv]
        return orig(argv, **kw)

    patched._ldw_patched = True
    bu.run_command = patched


def _build(reps: int | None = None):
    import concourse.bass as bass  # noqa: F401  (engine classes registered)
    import concourse.mybir as mybir
    import concourse.tile as tile
    from concourse import bacc

    # NOTE: the baseline's _patch_ldw_opt (--enable-ldw-opt=true) is NOT used:
    # bf16 matmuls get explicit InstLdweights from bass legalization, which
    # walrus rejects under ldw-opt ("InstLdweights is not compatible").

    f32 = mybir.dt.float32
    bf16 = mybir.dt.bfloat16
    add = mybir.AluOpType.add
    sub = mybir.AluOpType.subtract

    nc = bacc.Bacc("TRN2", target_bir_lowering=False, debug=False,
                   num_devices=N_CORES)
    x_d = nc.declare_dram_parameter("x", [N_PER_CORE, C_IN, HP * WP], bf16,
                                    isOutput=False)
    # Winograd weights: [ci, (p,kw,cob) flattened, co] -> [128, 24*128]
    w_d = nc.declare_dram_parameter("w", [C_IN, 4 * 3 * 2 * 128], bf16,
                                    isOutput=False)
    # cols: [bias_cob0, bias_cob1, 0, 0] — zero cols give every Act drain op
    # the same Identity+bias form (single activation table)
    b_d = nc.declare_dram_parameter("b", [128, 4], f32, isOutput=False)
    o_d = nc.declare_dram_parameter("o", [N_PER_CORE, 2, 128, H * W], bf16,
                                    isOutput=True)

    def widx(p, kw, cob):
        return ((p * 3 + kw) * 2 + cob) * 128

    with tile.TileContext(nc) as tc:
        with (
            tc.tile_pool(name="const", bufs=1) as const_pool,
            tc.tile_pool(name="xp", bufs=2) as x_pool,
            tc.tile_pool(name="vp", bufs=2) as v_pool,
            tc.tile_pool(name="cp", bufs=8) as c_pool,
            tc.tile_pool(name="op", bufs=3) as o_pool,
            tc.tile_pool(name="ps", bufs=8, space="PSUM") as psum_pool,
        ):
            b_sb = const_pool.tile([128, 4], f32, tag="b")
            w_sb = const_pool.tile([C_IN, 4 * 3 * 2 * 128], bf16, tag="w")
            x_first = x_pool.tile([C_IN, HP * WP], bf16, tag="x")
            # interleave weight chunks with the first-image load in the order
            # the first matmuls consume them

            nc.sync.dma_start(w_sb[:, 0:768], w_d[:, 0:768])
            nc.sync.dma_start(x_first[:, 0:29 * WP], x_d[0, :, 0:29 * WP])
            nc.sync.dma_start(b_sb[:], b_d[:])
            nc.sync.dma_start(w_sb[:, 768:2048], w_d[:, 768:2048])
            nc.sync.dma_start(x_first[:, 29 * WP:], x_d[0, :, 29 * WP:])
            nc.sync.dma_start(w_sb[:, 2048:], w_d[:, 2048:])

            def body():
                for n in range(N_PER_CORE):
                    if n == 0:
                        x_sb = x_first
                    else:
                        x_sb = x_pool.tile([C_IN, HP * WP], bf16, tag="x")
                        nc.sync.dma_start(x_sb[:, 0:29 * WP],
                                          x_d[n, :, 0:29 * WP])
                        nc.sync.dma_start(x_sb[:, 29 * WP:],
                                          x_d[n, :, 29 * WP:])
                    x3 = x_sb[:].rearrange("p (h w) -> p h w", w=WP)
                    d0 = x3[:, 0:2 * TR:2, :]
                    d1 = x3[:, 1:2 * TR:2, :]
                    d2 = x3[:, 2:2 * TR + 1:2, :]
                    d3 = x3[:, 3:2 * TR + 2:2, :]
                    v = [v_pool.tile([C_IN, VW], bf16, tag=f"v{p}",
                                     name=f"v{p}") for p in range(4)]
                    v3 = [t[:].rearrange("p (h w) -> p h w", w=WP) for t in v]
                    # input transform: DVE takes 3 points, GpSimd one
                    nc.gpsimd.tensor_tensor(v3[0][:], d0, d2, sub)
                    nc.vector.tensor_tensor(v3[1][:], d1, d2, add)
                    nc.vector.tensor_tensor(v3[2][:], d2, d1, sub)
                    nc.vector.tensor_tensor(v3[3][:], d1, d3, sub)
                    for cob in range(2):
                        bias = b_sb[:, cob:cob + 1]
                        zero = b_sb[:, 2:3]
                        for tg in range(N_CHUNK):
                            t0r = tg * TG
                            M = {}

                            def mm(p):
                                M[p] = psum_pool.tile([128, NF], f32,
                                                      tag="ps", name=f"m{p}")
                                for kw in range(K):
                                    c0 = widx(p, kw, cob)
                                    nc.tensor.matmul(
                                        M[p][:], w_sb[:, c0:c0 + 128],
                                        v3[p][:, t0r:t0r + TG, kw:kw + W],
                                        start=(kw == 0), stop=(kw == K - 1),
                                    )

                            o_sb = o_pool.tile([128, 2 * NF], bf16, tag="o")
                            o3 = o_sb[:].rearrange("p (h w) -> p h w", w=W)
                            # drain: Act evacuates M1/M2/M3 (GpSimd cannot
                            # read PSUM); DVE: t1=c1-c2, t0=M0+c1, y0=t0+c2;
                            # GpSimd: y1=t1-c3 (all-SBUF)
                            mm(1)
                            c1 = c_pool.tile([128, NF], bf16, tag="c1")
                            nc.scalar.add(c1[:], M[1][:], bias)
                            mm(2)
                            c2 = c_pool.tile([128, NF], bf16, tag="c2")
                            nc.scalar.add(c2[:], M[2][:], zero)
                            t1 = c_pool.tile([128, NF], bf16, tag="t1")
                            nc.vector.tensor_tensor(t1[:], c1[:], c2[:], sub)
                            mm(3)
                            c3 = c_pool.tile([128, NF], bf16, tag="c3")
                            nc.scalar.add(c3[:], M[3][:], zero)
                            nc.gpsimd.tensor_tensor(
                                o3[:, 1:2 * TG:2, :], t1[:].rearrange(
                                    "p (h w) -> p h w", w=W),
                                c3[:].rearrange("p (h w) -> p h w", w=W),
                                sub)
                            mm(0)
                            t0 = c_pool.tile([128, NF], bf16, tag="t0")
                            nc.vector.tensor_tensor(t0[:], M[0][:], c1[:], add)
                            nc.vector.tensor_tensor(
                                o3[:, 0:2 * TG:2, :], t0[:].rearrange(
                                    "p (h w) -> p h w", w=W),
                                c2[:].rearrange("p (h w) -> p h w", w=W), add)
                            nc.sync.dma_start(
                                o_d[n, cob][:, 2 * NF * tg:2 * NF * (tg + 1)],
                                o_sb[:],
                            )

            if reps is None:
                body()
            else:
                with tc.For_i(0, reps, 1):
                    body()

    nc.compile()
    return nc


def prep_inputs(x: np.ndarray, weight: np.ndarray, bias: np.ndarray):
    """Host-side layout prep shared by kernel() and the timing harness."""
    import ml_dtypes

    x = np.asarray(x, dtype=np.float32)
    weight = np.asarray(weight, dtype=np.float32)
    bias = np.asarray(bias, dtype=np.float32)

    xp = np.pad(x, ((0, 0), (0, 0), (1, 1), (1, 1)))          # (32,128,58,58)
    xp = xp.astype(ml_dtypes.bfloat16)
    xp = xp.reshape(N_CORES, N_PER_CORE, C_IN, HP * WP)

    # Winograd weight transform over kh (fp32, then bf16)
    g0, g1, g2 = weight[:, :, 0, :], weight[:, :, 1, :], weight[:, :, 2, :]
    U = np.stack([g0, (g0 + g1 + g2) * 0.5, (g0 - g1 + g2) * 0.5, g2])
    # U: (p 4, co 256, ci 128, kw 3) -> [ci, p, kw, cob, co128] -> [128, 3072]
    U = U.reshape(4, 2, 128, C_IN, K).transpose(3, 0, 4, 1, 2)
    wr = np.ascontiguousarray(U).reshape(C_IN, 4 * K * 2 * 128)
    wr = wr.astype(ml_dtypes.bfloat16)

    br = np.zeros((128, 4), dtype=np.float32)                  # [128, 4]
    br[:, 0:2] = bias.reshape(2, 128).T

    return [
        {"x": np.ascontiguousarray(xp[c]), "w": wr, "b": br}
        for c in range(N_CORES)
    ]


def kernel(x: np.ndarray, weight: np.ndarray, bias: np.ndarray) -> np.ndarray:
    from concourse.bass_utils import run_bass_kernel_spmd

    if "nc" not in _compiled:
        _compiled["nc"] = _build()
    nc = _compiled["nc"]

    in_maps = prep_inputs(x, weight, bias)
    res = run_bass_kernel_spmd(nc, in_maps, list(range(N_CORES)))
    out = np.stack([np.asarray(r["o"], dtype=np.float32)
                    for r in res.results])                     # (8,4,2,128,3136)
    out = out.reshape(N_CORES * N_PER_CORE, C_OUT, H, W)
    return out
